# revision 7
# baseline (speedup 1.0000x reference)
"""Block-sparse DSD matmul  y = x @ W^T  on 8 TRN2 NeuronCores.

x: [2048, 4096] f32, W given as 2048 sparse 32x32 blocks at (rows, cols)
block coordinates in a 128x128 block grid. y: [2048, 4096] f32.

Strategy (batch-parallel SPMD, identical program on 8 cores):
  - Shard batch 8 ways (256 rows/core); the sparse structure is identical
    on every core so one SPMD program works with per-core x shards.
  - All tensors cast to bf16 on host: PE matmuls run 1-pass and HBM
    traffic halves.  PSUM accumulates f32; y is written back bf16.
  - Compute y^T tiles on-chip: for block (r, c):
        y^T[32r:32r+32, :] += W_blk @ x^T[32c:32c+32, :]
    As a PE matmul: out = lhsT.T @ rhs with lhsT = W_blk^T (stationary,
    32x32), rhs = x^T chunk [32, 256].
  - 16-way 32x32 PE subarray tiling: lane a = c%4 picks the SBUF
    partition strip (PE row group); row-blocks are packed 4 to a "group",
    strip b picks the PSUM partition strip (PE col group).  Each lane
    accumulates into its own PSUM bank; banks fold via ACT/DVE/Pool.
  - DECOUPLED WEIGHT LOADS: instead of bass's fused matmul (whose
    implicit LDWEIGHTS->MATMUL dependency costs ~30ns of sequencer
    dispatch per slot), emit a standalone LDWEIGHTS stream running
    LOOKAHEAD slots ahead of the ldweights=False MATMUL stream.  Each
    MM's weights are long since loaded when it issues, so both
    instructions dispatch back-to-back and the pace is set by the
    subarray drain (~22ns/slot over 16 subarrays) instead of the
    dependency chain (~35ns/slot).
  - Weights are packed per lane strip (no group-alignment padding) and
    slots are emitted in near-uniform b round-robin so a subarray is
    revisited only every ~16 slots.
  - DMA: x in 4 chunks + w in 8 chunks (4 groups each, 4 strip DMAs) on
    the Sync HWDGE queue; y^T written per 4 groups on the Scalar HWDGE
    queue so output never queues behind input loads.
  - Host: pre-transposes x into partition-major per-core layout, packs
    transposed weight blocks into a lane-major array, assembles y.
"""

import numpy as np
import ml_dtypes

BF16 = ml_dtypes.bfloat16

# toggles used by test.py only; harness uses defaults
_RUN = {"trace": False, "trace_cores": [0], "last": None}

B, K, OUT, BLK, NNZ = 2048, 4096, 4096, 32, 2048
NCORES = 8
BC = B // NCORES          # 256 batch rows per core
NT = K // 128             # 32 x^T partition-tiles
NRB = OUT // BLK          # 128 row blocks
NG = NRB // 4             # 32 groups of 4 row blocks

TSORT_G = 8               # groups with t-monotonic slot order (x streaming)
LOOKAHEAD = 3             # LDWEIGHTS slots ahead of MATMUL stream
WCG = 4                   # groups per w chunk
XCH = 4                   # x chunks


def _build_schedule(w, rows, cols):
    """Group assignment + per-(group, lane) slot schedule + packed weights."""
    cnt = np.bincount(rows, minlength=NRB)
    order = np.argsort(-cnt, kind="stable")
    rmap = np.empty((NG, 4), dtype=np.int64)
    for rank, r in enumerate(order):
        rnd, pos = rank // NG, rank % NG
        g = pos if rnd % 2 == 0 else NG - 1 - pos
        rmap[g, rnd] = r

    gb_of_row = {}
    for g in range(NG):
        for b in range(4):
            gb_of_row[int(rmap[g, b])] = (g, b)

    cells = [[[[] for _ in range(4)] for _ in range(4)] for _ in range(NG)]
    for n in range(NNZ):
        g, b = gb_of_row[int(rows[n])]
        cells[g][int(cols[n]) % 4][b].append(n)

    prog = []
    wts = [[] for _ in range(4)]   # per-lane packed W^T blocks, in slot order
    lane_off = []
    for g in range(NG):
        lanes = []
        offs_g = []
        for a in range(4):
            raw = []
            for b in range(4):
                cl = cells[g][a][b]
                if not cl:
                    raw.append((0, b, np.zeros((BLK, BLK), np.float32)))
                for n in cl:
                    raw.append((int(cols[n]) // 4, b,
                                np.ascontiguousarray(w[n].T)))
            if g < TSORT_G:
                # x still streaming: keep t-monotonic, greedily pick the
                # least-recently-used b within a small t window
                raw.sort(key=lambda s: s[0])
                reordered = []
                pend = list(raw)
                recent = []
                while pend:
                    pick = 0
                    best = -1
                    for j in range(min(8, len(pend))):
                        if pend[j][0] > pend[0][0] + 3:
                            break
                        d = (len(recent) - recent[::-1].index(pend[j][1])
                             if pend[j][1] in recent else 99)
                        if d > best:
                            best = d
                            pick = j
                        if d == 99:
                            break
                    slot = pend.pop(pick)
                    reordered.append(slot)
                    recent.append(slot[1])
                    if len(recent) > 3:
                        recent.pop(0)
                raw = reordered
            else:
                # x resident: rate-balanced b round-robin
                byb = [[s for s in raw if s[1] == b] for b in range(4)]
                for lst in byb:
                    lst.sort(key=lambda s: s[0])
                sched = []
                done = [0] * 4
                for _ in range(len(raw)):
                    best_b, best_v = -1, 1e9
                    for b in range(4):
                        if done[b] < len(byb[b]):
                            v = (done[b] + 0.5) / len(byb[b])
                            if v < best_v:
                                best_v, best_b = v, b
                    sched.append(byb[best_b][done[best_b]])
                    done[best_b] += 1
                raw = sched
            first = {}
            last = {}
            for i, (_, b, _) in enumerate(raw):
                first.setdefault(b, i)
                last[b] = i
            slots = [(t, b, i == first[b], i == last[b])
                     for i, (t, b, _) in enumerate(raw)]
            lanes.append(slots)
            offs_g.append(len(wts[a]))
            for _, _, wt in raw:
                wts[a].append(wt)
        prog.append(lanes)
        lane_off.append(offs_g)

    lane_len = [len(wts[a]) for a in range(4)]
    tot = max(lane_len)
    wpk = np.zeros((128, tot * BLK), dtype=np.float32)
    for a in range(4):
        for idx, wt in enumerate(wts[a]):
            wpk[32 * a:32 * a + 32, idx * BLK:(idx + 1) * BLK] = wt
    return prog, lane_off, lane_len, tot, wpk, rmap


def _matmul_noload(nc, mybir, out, lhsT, rhs, start, stop, tile_position):
    """InstMatmult with ldweights=False: uses the stationary weights
    already loaded into the subarray by a prior standalone LDWEIGHTS."""
    te = nc.tensor
    ifmap_ap = te.lower_ap(rhs.opt({0}), opt=False)
    weights_ap = te.lower_ap(lhsT.opt({0}), opt=False, for_matmul_weights=True)
    out_ap = te.lower_ap(out)
    return te.add_instruction(
        mybir.InstMatmult(
            name=te.bass.get_next_instruction_name(),
            replication_resolution=0,
            replication_shift_amnt=0,
            replication_num_rows=0,
            start_tensor_calc=start,
            stop_tensor_calc=stop,
            ins=[ifmap_ap, weights_ap],
            outs=[out_ap],
            perf_mode=None,
            is_transpose=None,
            ifmap_quant_offset=None,
            weights_quant_offset=None,
            bass_skip_group_check=False,
            tile_position=tile_position,
            tile_size=(32, 32),
            ldweights=False,
        )
    )


def kernel(x, w, rows, cols, out_blocks=None):
    import concourse.bass as bass
    import concourse.bacc as bacc
    import concourse.tile as tile
    import concourse.mybir as mybir
    from concourse.bass_utils import run_bass_kernel_spmd
    from contextlib import ExitStack

    x = np.asarray(x, dtype=np.float32)
    w = np.asarray(w, dtype=np.float32)
    rows = np.asarray(rows).astype(np.int64)
    cols = np.asarray(cols).astype(np.int64)

    prog, lane_off, lane_len, tot, wpk, rmap = _build_schedule(w, rows, cols)
    wpk16 = wpk.astype(BF16)

    # x^T, per-core partition-major: xarr[core, p, t*BC + j] = x[BC*core + j, 128*t + p]
    xarr = np.ascontiguousarray(
        x.reshape(NCORES, BC, NT, 128).transpose(0, 3, 2, 1)
    ).reshape(NCORES, 128, NT * BC).astype(BF16)

    f32 = mybir.dt.float32
    bf16 = mybir.dt.bfloat16
    nc = bacc.Bacc()
    xt_d = nc.declare_dram_parameter("xt", [128, NT * BC], bf16, isOutput=False)
    wp_d = nc.declare_dram_parameter("wpk", [128, tot * BLK], bf16, isOutput=False)
    yt_d = nc.declare_dram_parameter("yt", [128, NG * BC], bf16, isOutput=True)

    with tile.TileContext(nc) as tc, ExitStack() as ctx:
        xp = ctx.enter_context(tc.tile_pool(name="x", bufs=1))
        wpool = ctx.enter_context(tc.tile_pool(name="w", bufs=1))
        pp = ctx.enter_context(tc.tile_pool(name="ps", bufs=8, space="PSUM"))
        tp = ctx.enter_context(tc.tile_pool(name="tmp", bufs=3))
        yp = ctx.enter_context(tc.tile_pool(name="y", bufs=2))

        nwc = NG // WCG
        wtiles = {}

        def load_w(k):
            g0, g1 = k * WCG, (k + 1) * WCG
            exts = []
            for a in range(4):
                lo = lane_off[g0][a]
                hi = lane_off[g1][a] if g1 < NG else lane_len[a]
                exts.append((lo, hi))
            width = max(hi - lo for lo, hi in exts)
            wsb = wpool.tile([128, width * BLK], bf16, tag=f"w{k}",
                             name=f"w{k}")
            for a in range(4):
                lo, hi = exts[a]
                if hi > lo:
                    nc.sync.dma_start(
                        wsb[32 * a:32 * a + 32, :(hi - lo) * BLK],
                        wp_d[32 * a:32 * a + 32, lo * BLK:hi * BLK])
            wtiles[k] = (wsb, exts)

        XC = NT // XCH
        xts = []

        def load_x(ci):
            xc = xp.tile([128, XC * BC], bf16, tag=f"xc{ci}", name=f"xc{ci}")
            nc.sync.dma_start(
                xc[:], xt_d[:, ci * XC * BC:(ci + 1) * XC * BC])
            xts.append(xc)

        # DMA ring is FIFO: first w chunk and x lead the queue.
        load_w(0)
        load_x(0)
        load_w(1)
        load_x(1)
        load_w(2)
        for ci in range(2, XCH):
            load_x(ci)
        load_w(3)

        def rhs_of(t):
            return xts[t // XC][:, (t % XC) * BC:(t % XC + 1) * BC]

        # Flatten the whole schedule into one slot list so the LDWEIGHTS
        # stream can run LOOKAHEAD slots ahead across group boundaries.
        # Each entry: (g, a, b, t, start, stop, wcol_slice, kchunk)
        sched = []
        group_end = []                     # index into sched after group g
        for g in range(NG):
            k = g // WCG
            n_g = max(len(prog[g][a]) for a in range(4))
            for idx in range(n_g):
                for a in range(4):
                    if idx < len(prog[g][a]):
                        t, b, st, sp, = prog[g][a][idx]
                        sched.append((g, a, b, t, st, sp, idx, k))
            group_end.append(len(sched))

        nslots = len(sched)
        ps_of_g = {}

        def w_ap(s):
            g, a, b, t, st, sp, idx, k = sched[s]
            wsb, exts = wtiles[k]
            wcol = (lane_off[g][a] - exts[a][0] + idx) * BLK
            return wsb[32 * a:32 * a + 32, wcol:wcol + BLK]

        def emit_ldw(s):
            g, a, b, t, st, sp, idx, k = sched[s]
            nc.tensor.ldweights(w_ap(s), tile_position=(32 * a, 32 * b))

        def emit_mm(s):
            g, a, b, t, st, sp, idx, k = sched[s]
            ps = ps_of_g[g]
            _matmul_noload(
                nc, mybir,
                ps[a][32 * b:32 * b + 32, :],
                w_ap(s),
                rhs_of(t)[32 * a:32 * a + 32, :],
                st, sp, (32 * a, 32 * b),
            )

        y4 = None
        g_next = 0                      # next group needing psum alloc
        gdone = 0                       # next group to fold
        for s in range(nslots + LOOKAHEAD):
            # allocate psum for groups as their first LDW approaches
            if s < nslots:
                g, a, b, t, st, sp, idx, k = sched[s]
                if g >= g_next:
                    nk = k + 4
                    if g % WCG == 0 and nk < nwc and nk not in wtiles:
                        load_w(nk)
                    ps_of_g[g] = [pp.tile([128, BC], f32, tag="ps",
                                          name=f"ps{g}_{a2}")
                                  for a2 in range(4)]
                    g_next = g + 1
                emit_ldw(s)
            if s >= LOOKAHEAD:
                emit_mm(s - LOOKAHEAD)
                # fold any group whose matmuls all issued
                while gdone < NG and s - LOOKAHEAD + 1 == group_end[gdone]:
                    g2 = gdone
                    ps = ps_of_g.pop(g2)
                    s0 = tp.tile([128, BC], f32, tag="t0")
                    nc.scalar.copy(s0[:], ps[0][:])
                    s2 = tp.tile([128, BC], f32, tag="t1")
                    nc.scalar.copy(s2[:], ps[2][:])
                    a01 = tp.tile([128, BC], f32, tag="t2")
                    nc.vector.tensor_add(a01[:], s0[:], ps[1][:])
                    a23 = tp.tile([128, BC], f32, tag="t3")
                    nc.vector.tensor_add(a23[:], s2[:], ps[3][:])
                    if g2 % 4 == 0:
                        y4 = yp.tile([128, 4 * BC], bf16, tag="y")
                    nc.gpsimd.tensor_add(
                        y4[:, (g2 % 4) * BC:(g2 % 4 + 1) * BC],
                        a01[:], a23[:])
                    if g2 == NG - 2:
                        nc.scalar.dma_start(
                            yt_d[:, (g2 - 2) * BC:(g2 + 1) * BC],
                            y4[:, :3 * BC])
                    elif g2 == NG - 1:
                        nc.scalar.dma_start(
                            yt_d[:, g2 * BC:(g2 + 1) * BC],
                            y4[:, 3 * BC:4 * BC])
                    elif g2 % 4 == 3:
                        nc.scalar.dma_start(
                            yt_d[:, (g2 - 3) * BC:(g2 + 1) * BC], y4[:])
                    gdone += 1

    nc.compile()

    in_maps = [{"xt": xarr[i], "wpk": wpk16} for i in range(NCORES)]
    res = run_bass_kernel_spmd(
        nc, in_maps, list(range(NCORES)),
        trace=_RUN["trace"], trace_cores=_RUN["trace_cores"],
    )
    _RUN["last"] = res

    feat = np.empty(OUT, dtype=np.int64)
    for g in range(NG):
        for b in range(4):
            feat[128 * g + 32 * b:128 * g + 32 * b + 32] = \
                32 * rmap[g, b] + np.arange(32)

    y = np.empty((B, OUT), dtype=np.float32)
    for i in range(NCORES):
        ytp = np.asarray(res.results[i]["yt"]).astype(np.float32)
        ytp = ytp.reshape(128, NG, BC).transpose(1, 0, 2).reshape(OUT, BC)
        yT = np.empty((OUT, BC), dtype=np.float32)
        yT[feat] = ytp
        y[BC * i:BC * (i + 1), :] = yT.T
    return y


# revision 23
# speedup vs baseline: 1.1605x; 1.1605x over previous
"""Block-sparse DSD matmul  y = x @ W^T  on 8 TRN2 NeuronCores.

x: [2048, 4096] f32, W given as 2048 sparse 32x32 blocks at (rows, cols)
block coordinates in a 128x128 block grid. y: [2048, 4096] f32.

Strategy (batch-parallel SPMD, identical program on 8 cores):
  - Shard batch 8 ways (256 rows/core); one SPMD program, per-core x.
  - All tensors bf16 on host: 1-pass PE matmuls, half the HBM traffic.
    PSUM accumulates f32; y written back bf16.
  - 16-way 32x32 PE subarray tiling: lane a (col-block -> partition
    strip, host-chosen) is the PE row group; row-blocks pack 4 to a
    "group", strip b is the PSUM partition strip.
  - UNITS: per (group, lane), blocks arrange into rounds of <=4 matmuls
    with distinct strips b; weights pack into [32,128] windows (block of
    strip b at column 32b).  Units per lane = max blocks in any cell; a
    host local search balances cells (col->lane / row->group swaps).
  - EMISSION uses ordinary fused matmuls (LDWEIGHTS+MATMUL pairs - the
    only form whose ordering the Tile scheduler preserves), rotating
    unit streams of two active group-pairs (16 streams) lane-major.
  - FUSION PASS (after Tile scheduling, before compile): the PE
    front-end costs ~30ns per LDWEIGHTS vs ~5ns per MATMUL, so for each
    unit whose lane has been quiet >=150ns (est.) the pass replaces the
    unit's first implicit LDWEIGHTS with one wide [32,128] load and
    deletes the other implicit loads (semaphore ops merge onto the
    following matmul).  Units too close to their lane's previous matmul
    keep the safe per-slot pairs (the PE does NOT interlock a weight
    load against in-flight matmuls on the same subarrays - reloading
    within ~120ns of a matmul issue corrupts its stream).
  - PSUM: groups pair up; each lane's bank holds both groups' [128,256]
    halves, folded per-pair with [128,512]-wide ops: ACT copies ps0/ps2,
    DVE adds ps1/ps3 (one PSUM operand per op), Pool does the SBUF-only
    final add + bf16 downcast.  8 banks = 2 pairs in flight.
  - DMA: x in 4 chunks + w per-strip chunks on the Sync HWDGE queue;
    y^T written per 4 groups on the Scalar HWDGE queue.
"""

import numpy as np
import ml_dtypes

BF16 = ml_dtypes.bfloat16

# toggles used by test.py only; harness uses defaults
_RUN = {"trace": False, "trace_cores": [0], "last": None}

B, K, OUT, BLK, NNZ = 2048, 4096, 4096, 32, 2048
NCORES = 8
BC = B // NCORES          # 256 batch rows per core
NT = K // 128             # 32 x^T partition-tiles
NCB = K // BLK            # 128 col blocks
NRB = OUT // BLK          # 128 row blocks
NG = NRB // 4             # 32 groups of 4 row blocks
NP = NG // 2              # 16 group pairs

WCG = 4                   # groups per w chunk
XCH = 4                   # x chunks
SWEEPS = 3                # local-search sweeps for cell balancing
SPACING = 150.0           # est. ns a wide load must trail its lane's last MM
T_LDW, T_MM, T_OTH = 35.0, 5.0, 10.0


def _balance(rows, cols, seed=0):
    """Assign cols->lanes (32 each) and rows->(group, strip) minimizing
    total units = sum over (g, lane) of max cell count."""
    rng = np.random.default_rng(seed)
    cnt_r = np.bincount(rows, minlength=NRB)
    order = np.argsort(-cnt_r, kind="stable")
    g_of = np.empty(NRB, np.int64)
    b_of = np.empty(NRB, np.int64)
    for rank, r in enumerate(order):
        rnd, pos = rank // NG, rank % NG
        g = pos if rnd % 2 == 0 else NG - 1 - pos
        g_of[r], b_of[r] = g, rnd

    cnt_c = np.bincount(cols, minlength=NCB)
    corder = np.argsort(-cnt_c, kind="stable")
    lane_of = np.empty(NCB, np.int64)
    for rank, c in enumerate(corder):
        rnd, pos = rank // 4, rank % 4
        lane_of[c] = pos if rnd % 2 == 0 else 3 - pos

    rows_of_c = [[] for _ in range(NCB)]
    cols_of_r = [[] for _ in range(NRB)]
    for n in range(NNZ):
        rows_of_c[int(cols[n])].append(int(rows[n]))
        cols_of_r[int(rows[n])].append(int(cols[n]))

    cnt = np.zeros((NG, 4, 4), np.int64)
    for n in range(NNZ):
        r, c = int(rows[n]), int(cols[n])
        cnt[g_of[r], lane_of[c], b_of[r]] += 1

    lmax = cnt.max(axis=2)

    def try_col_swap(c1, c2):
        a1, a2 = lane_of[c1], lane_of[c2]
        if a1 == a2:
            return False
        touched = set()
        for r in rows_of_c[c1]:
            cnt[g_of[r], a1, b_of[r]] -= 1
            cnt[g_of[r], a2, b_of[r]] += 1
            touched.add(int(g_of[r]))
        for r in rows_of_c[c2]:
            cnt[g_of[r], a2, b_of[r]] -= 1
            cnt[g_of[r], a1, b_of[r]] += 1
            touched.add(int(g_of[r]))
        delta = 0.0
        for g in touched:
            delta += (max(1, cnt[g, a1].max()) - max(1, lmax[g, a1])
                      + max(1, cnt[g, a2].max()) - max(1, lmax[g, a2]))
        if delta < 0:
            lane_of[c1], lane_of[c2] = a2, a1
            for g in touched:
                lmax[g, a1] = cnt[g, a1].max()
                lmax[g, a2] = cnt[g, a2].max()
            return True
        for r in rows_of_c[c1]:
            cnt[g_of[r], a1, b_of[r]] += 1
            cnt[g_of[r], a2, b_of[r]] -= 1
        for r in rows_of_c[c2]:
            cnt[g_of[r], a2, b_of[r]] += 1
            cnt[g_of[r], a1, b_of[r]] -= 1
        return False

    def try_row_swap(r1, r2):
        g1, b1 = int(g_of[r1]), int(b_of[r1])
        g2, b2 = int(g_of[r2]), int(b_of[r2])
        if g1 == g2:
            return False
        for c in cols_of_r[r1]:
            cnt[g1, lane_of[c], b1] -= 1
            cnt[g2, lane_of[c], b2] += 1
        for c in cols_of_r[r2]:
            cnt[g2, lane_of[c], b2] -= 1
            cnt[g1, lane_of[c], b1] += 1
        delta = 0.0
        for g in (g1, g2):
            for a in range(4):
                delta += max(1, cnt[g, a].max()) - max(1, lmax[g, a])
        if delta < 0:
            g_of[r1], b_of[r1] = g2, b2
            g_of[r2], b_of[r2] = g1, b1
            for g in (g1, g2):
                lmax[g] = cnt[g].max(axis=1)
            return True
        for c in cols_of_r[r1]:
            cnt[g1, lane_of[c], b1] += 1
            cnt[g2, lane_of[c], b2] -= 1
        for c in cols_of_r[r2]:
            cnt[g2, lane_of[c], b2] += 1
            cnt[g1, lane_of[c], b1] -= 1
        return False

    for _ in range(SWEEPS):
        cs = rng.permutation(NCB)
        for i in range(0, NCB - 1, 2):
            try_col_swap(int(cs[i]), int(cs[i + 1]))
        for c1 in range(NCB):
            try_col_swap(c1, int(rng.integers(NCB)))
        rs = rng.permutation(NRB)
        for i in range(0, NRB - 1, 2):
            try_row_swap(int(rs[i]), int(rs[i + 1]))
        for r1 in range(NRB):
            try_row_swap(r1, int(rng.integers(NRB)))

    rmap = np.empty((NG, 4), np.int64)
    for r in range(NRB):
        rmap[g_of[r], b_of[r]] = r
    return g_of, b_of, lane_of, rmap


def _build_schedule(w, rows, cols):
    g_of, b_of, lane_of, rmap = _balance(rows, cols)

    t_of = np.empty(NCB, np.int64)
    lanes_cols = [[] for _ in range(4)]
    for c in range(NCB):
        lanes_cols[lane_of[c]].append(c)
    for a in range(4):
        assert len(lanes_cols[a]) == NT
        for t, c in enumerate(sorted(lanes_cols[a])):
            t_of[c] = t

    cells = [[[[] for _ in range(4)] for _ in range(4)] for _ in range(NG)]
    for n in range(NNZ):
        r, c = int(rows[n]), int(cols[n])
        cells[int(g_of[r])][int(lane_of[c])][int(b_of[r])].append(
            (int(t_of[c]), n))
    for g in range(NG):
        for a in range(4):
            for b in range(4):
                cells[g][a][b].sort()

    # Every unit carries exactly 4 matmuls (b = 0..3): cells shorter than
    # the lane's round count run zero-weight pads (wpk is zero there), so
    # each unit's weight slices are contiguous [32,128] windows and the
    # per-cell accumulate chain spans all rounds (start at j=0, stop at
    # the last round).
    units = [[None] * 4 for _ in range(NG)]
    lane_rounds = np.zeros((NG, 4), np.int64)
    for g in range(NG):
        for a in range(4):
            r_ga = max(1, max(len(cells[g][a][b]) for b in range(4)))
            lane_rounds[g, a] = r_ga
            rd = []
            for j in range(r_ga):
                row = []
                for b in range(4):
                    cl = cells[g][a][b]
                    t = cl[j][0] if j < len(cl) else 0
                    row.append((b, t, j == 0, j == r_ga - 1))
                rd.append(row)
            units[g][a] = rd

    offa = np.zeros((NG + 1, 4), np.int64)
    for g in range(NG):
        offa[g + 1] = offa[g] + lane_rounds[g]
    width = int(offa[NG].max())
    wpk = np.zeros((128, width * 128), dtype=np.float32)
    for g in range(NG):
        for a in range(4):
            for j in range(int(lane_rounds[g, a])):
                base = (int(offa[g, a]) + j) * 128
                for b in range(4):
                    cl = cells[g][a][b]
                    if j < len(cl):
                        _, n = cl[j]
                        wpk[32 * a:32 * a + 32,
                            base + 32 * b:base + 32 * b + 32] = w[n].T
    return units, lane_rounds, offa, width, wpk, rmap, lane_of, t_of


def _fuse_units(nc, mybir, unit_recs):
    """Post-scheduler pass: replace each fusible unit's first implicit
    LDWEIGHTS with one wide [32,128] load, delete the unit's other
    implicit loads.  Skips units whose lane had a matmul < SPACING est-ns
    ago (no hardware interlock on weight reloads)."""
    first_of = {}
    member = {}
    for rec in unit_recs:
        if len(rec["mms"]) < 2:
            continue
        first_of[id(rec["mms"][0])] = rec
        for m in rec["mms"]:
            member[id(m)] = rec

    for blk in nc.m.functions[0].blocks:
        insts = list(blk.instructions)
        n = len(insts)
        out = []
        pend = []
        t = 0.0
        lane_last_mm = [-1e9] * 4
        fused = set()
        for idx, ins in enumerate(insts):
            if (isinstance(ins, mybir.InstLdweights)
                    and ins.tile_size == (32, 32)):
                nxt = insts[idx + 1] if idx + 1 < n else None
                rec = (member.get(id(nxt))
                       if isinstance(nxt, mybir.InstMatmult) else None)
                if rec is not None:
                    if (id(rec["mms"][0]) == id(nxt)
                            and id(rec) not in fused
                            and t - lane_last_mm[rec["a"]] >= SPACING):
                        # widen the resolved [32,32] AP of this implicit
                        # load to the unit's [32,128] window: the first
                        # MM's weights sit at column 32*b0 of the window.
                        src = ins.ins[0]
                        b0 = rec["b0"]
                        wap = mybir.PhysicalAccessPattern(
                            ap=[list(src.ap[0]), [1, 128]],
                            offset=src.offset - 32 * b0,
                            dtype=src.dtype,
                            memref=src.memref,
                            memsetref=src.memsetref,
                        )
                        wl = mybir.InstLdweights(
                            name=nc.get_next_instruction_name(),
                            ins=[wap], outs=[],
                            tile_position=(32 * rec["a"], 0),
                            tile_size=(32, 128))
                        wl.engine = ins.engine
                        wl.sync_info = ins.sync_info
                        nc.register_instruction(wl)
                        out.append(wl)
                        t += T_LDW
                        fused.add(id(rec))
                        continue
                    if id(rec) in fused:
                        if ins.sync_info is not None:
                            pend.append(ins.sync_info)
                        continue
                out.append(ins)
                t += T_LDW
                continue
            if pend:
                si = ins.sync_info
                if si is None:
                    si = mybir.SyncInfo(on_wait=[], on_update=[])
                for p in pend:
                    si.on_wait = list(si.on_wait) + list(p.on_wait)
                    si.on_update = list(si.on_update) + list(p.on_update)
                ins.sync_info = si
                pend = []
            if isinstance(ins, mybir.InstMatmult):
                rec = member.get(id(ins))
                if rec is not None:
                    lane_last_mm[rec["a"]] = t
                t += T_MM
            else:
                t += T_OTH
            out.append(ins)
        assert not pend
        if len(out) != len(insts):
            blk.instructions[:] = out
        if fused:
            import sys
            print(f"fuse_units: {len(fused)}/{len(unit_recs)} units fused",
                  file=sys.stderr)


def kernel(x, w, rows, cols, out_blocks=None):
    import concourse.bass as bass
    import concourse.bacc as bacc
    import concourse.tile as tile
    import concourse.mybir as mybir
    from concourse.bass_utils import run_bass_kernel_spmd
    from contextlib import ExitStack

    x = np.asarray(x, dtype=np.float32)
    w = np.asarray(w, dtype=np.float32)
    rows = np.asarray(rows).astype(np.int64)
    cols = np.asarray(cols).astype(np.int64)

    (units, lane_rounds, offa, width, wpk, rmap, lane_of, t_of) = \
        _build_schedule(w, rows, cols)
    wpk16 = wpk.astype(BF16)

    perm = np.empty(NCB, np.int64)
    for c in range(NCB):
        perm[int(lane_of[c]) * NT + int(t_of[c])] = c
    xr = x.reshape(NCORES, BC, NCB, BLK)
    xarr = np.ascontiguousarray(
        xr[:, :, perm, :]
        .reshape(NCORES, BC, 4, NT, BLK)
        .transpose(0, 2, 4, 3, 1)
    ).reshape(NCORES, 128, NT * BC).astype(BF16)

    f32 = mybir.dt.float32
    bf16 = mybir.dt.bfloat16
    nc = bacc.Bacc()
    xt_d = nc.declare_dram_parameter("xt", [128, NT * BC], bf16, isOutput=False)
    wp_d = nc.declare_dram_parameter("wpk", [128, width * 128], bf16,
                                     isOutput=False)
    yt_d = nc.declare_dram_parameter("yt", [128, NG * BC], bf16, isOutput=True)

    unit_recs = []

    with tile.TileContext(nc) as tc, ExitStack() as ctx:
        xp = ctx.enter_context(tc.tile_pool(name="x", bufs=1))
        wpool = ctx.enter_context(tc.tile_pool(name="w", bufs=1))
        pp = ctx.enter_context(tc.tile_pool(name="ps", bufs=8, space="PSUM"))
        tp = ctx.enter_context(tc.tile_pool(name="tmp", bufs=3))
        yp = ctx.enter_context(tc.tile_pool(name="y", bufs=2))

        nwc = NG // WCG
        wtiles = {}

        def load_w(k):
            g0, g1 = k * WCG, (k + 1) * WCG
            exts = [(int(offa[g0, a]), int(offa[g1, a])) for a in range(4)]
            wspan = max(hi - lo for lo, hi in exts)
            wsb = wpool.tile([128, wspan * 128], bf16, tag=f"w{k}",
                             name=f"w{k}")
            for a in range(4):
                lo, hi = exts[a]
                if hi > lo:
                    nc.sync.dma_start(
                        wsb[32 * a:32 * a + 32, :(hi - lo) * 128],
                        wp_d[32 * a:32 * a + 32, lo * 128:hi * 128])
            wtiles[k] = (wsb, exts)

        XC = NT // XCH
        xts = []

        def load_x(ci):
            xc = xp.tile([128, XC * BC], bf16, tag=f"xc{ci}", name=f"xc{ci}")
            nc.sync.dma_start(
                xc[:], xt_d[:, ci * XC * BC:(ci + 1) * XC * BC])
            xts.append(xc)

        load_w(0)
        load_x(0)
        load_w(1)
        load_x(1)
        load_w(2)
        for ci in range(2, XCH):
            load_x(ci)
        load_w(3)

        def rhs_of(t):
            return xts[t // XC][:, (t % XC) * BC:(t % XC + 1) * BC]

        ps_of_pair = {}
        y4 = None

        def open_pair(p):
            k = (2 * p) // WCG
            if 2 * p % WCG == 0:
                nk = k + 4
                if nk < nwc and nk not in wtiles:
                    load_w(nk)
            ps_of_pair[p] = [pp.tile([128, 512], f32, tag="ps",
                                     name=f"ps{p}_{a}")
                             for a in range(4)]

        def emit_unit(p, gg, a, j):
            g = 2 * p + gg
            k = g // WCG
            wsb, exts = wtiles[k]
            base = (int(offa[g, a]) - exts[a][0] + j) * 128
            ps = ps_of_pair[p]
            half = gg * 256
            mms = []
            for b, t, st, sp in units[g][a][j]:
                bi = nc.tensor.matmul(
                    ps[a][32 * b:32 * b + 32, half:half + 256],
                    lhsT=wsb[32 * a:32 * a + 32,
                             base + 32 * b:base + 32 * b + 32],
                    rhs=rhs_of(t)[32 * a:32 * a + 32, :],
                    start=st, stop=sp,
                    tile_position=(32 * a, 32 * b),
                )
                mms.append(bi.ins)
            unit_recs.append({"a": a, "mms": mms,
                              "b0": units[g][a][j][0][0]})

        def fold_pair(p):
            nonlocal y4
            ps = ps_of_pair.pop(p)
            s0 = tp.tile([128, 512], f32, tag="t0")
            nc.scalar.copy(s0[:], ps[0][:])
            s2 = tp.tile([128, 512], f32, tag="t1")
            nc.scalar.copy(s2[:], ps[2][:])
            a01 = tp.tile([128, 512], f32, tag="t2")
            nc.vector.tensor_add(a01[:], s0[:], ps[1][:])
            a23 = tp.tile([128, 512], f32, tag="t3")
            nc.vector.tensor_add(a23[:], s2[:], ps[3][:])
            if p % 2 == 0:
                y4 = yp.tile([128, 4 * BC], bf16, tag="y")
            nc.gpsimd.tensor_add(
                y4[:, (p % 2) * 512:(p % 2) * 512 + 512], a01[:], a23[:])
            g_hi = 2 * p + 1
            if p == NP - 2:
                nc.scalar.dma_start(
                    yt_d[:, (g_hi - 1) * BC:(g_hi + 1) * BC],
                    y4[:, 0:512])
            elif p == NP - 1:
                nc.scalar.dma_start(
                    yt_d[:, (g_hi - 1) * BC:(g_hi + 1) * BC],
                    y4[:, 512:1024])
            elif p % 2 == 1:
                nc.scalar.dma_start(
                    yt_d[:, (g_hi - 3) * BC:(g_hi + 1) * BC], y4[:])

        # deterministic rotation: lane-major round-robin over the unit
        # streams of the two oldest unfinished pairs; prefer the oldest
        # pair so it folds early and the window slides.
        nxt = {}
        rem = {}

        def start_pair(p):
            open_pair(p)
            nxt[p] = {(gg, a): 0 for gg in range(2) for a in range(4)}
            rem[p] = sum(int(lane_rounds[2 * p + gg, a])
                         for gg in range(2) for a in range(4))

        start_pair(0)
        if NP > 1:
            start_pair(1)
        done_upto = 0
        opened = min(2, NP)
        finished = set()
        lane_rr = 0
        while done_upto < NP:
            emitted = False
            for _ in range(4):
                a = lane_rr % 4
                lane_rr += 1
                cand = None
                for p in sorted(nxt.keys()):
                    for gg in range(2):
                        j = nxt[p][(gg, a)]
                        if j < int(lane_rounds[2 * p + gg, a]):
                            cand = (p, gg, a, j)
                            break
                    if cand:
                        break
                if cand:
                    p, gg, a, j = cand
                    emit_unit(p, gg, a, j)
                    nxt[p][(gg, a)] = j + 1
                    rem[p] -= 1
                    emitted = True
                    if rem[p] == 0:
                        finished.add(p)
                        del nxt[p]
                        del rem[p]
                        while done_upto in finished:
                            fold_pair(done_upto)
                            finished.discard(done_upto)
                            done_upto += 1
                            if opened < NP:
                                start_pair(opened)
                                opened += 1
                    break
            if not emitted and done_upto < NP and not nxt:
                break

    nc.compile()

    in_maps = [{"xt": xarr[i], "wpk": wpk16} for i in range(NCORES)]
    res = run_bass_kernel_spmd(
        nc, in_maps, list(range(NCORES)),
        trace=_RUN["trace"], trace_cores=_RUN["trace_cores"],
    )
    _RUN["last"] = res

    feat = np.empty(OUT, dtype=np.int64)
    for g in range(NG):
        for b in range(4):
            feat[128 * g + 32 * b:128 * g + 32 * b + 32] = \
                32 * rmap[g, b] + np.arange(32)

    y = np.empty((B, OUT), dtype=np.float32)
    for i in range(NCORES):
        ytp = np.asarray(res.results[i]["yt"]).astype(np.float32)
        ytp = ytp.reshape(128, NG, BC).transpose(1, 0, 2).reshape(OUT, BC)
        yT = np.empty((OUT, BC), dtype=np.float32)
        yT[feat] = ytp
        y[BC * i:BC * (i + 1), :] = yT.T
    return y


# revision 26
# speedup vs baseline: 1.5177x; 1.3078x over previous
"""Block-sparse DSD matmul  y = x @ W^T  on 8 TRN2 NeuronCores.

x: [2048, 4096] f32, W given as 2048 sparse 32x32 blocks at (rows, cols)
block coordinates in a 128x128 block grid. y: [2048, 4096] f32.

Strategy (batch-parallel SPMD, identical program on 8 cores):
  - Shard batch 8 ways (256 rows/core); the sparse structure is identical
    on every core so one SPMD program works with per-core x shards.
  - All tensors cast to bf16 on host: PE matmuls run 1-pass (fp32 ran
    LOW_HIGH 2-pass) and HBM traffic halves.  PSUM accumulates f32;
    y is written back bf16 and widened on host.
  - Compute y^T tiles on-chip: for block (r, c):
        y^T[32r:32r+32, :] += W_blk @ x^T[32c:32c+32, :]
    As a PE matmul: out = lhsT.T @ rhs with lhsT = W_blk^T (stationary,
    32x32), rhs = x^T chunk [32, 256].
  - 16-way 32x32 PE subarray tiling: lane a = c%4 picks the SBUF
    partition strip (and PE row group); row-blocks are packed 4 to a
    "group", strip b in the group picks the PSUM partition strip (PE col
    group).  Each lane accumulates into its own PSUM bank; the 4 lane
    banks fold via ACT (2 copies), DVE (2 adds) and Pool (final add +
    bf16 cast) so no single engine serializes the evacuation.
  - DMA: x in 4 chunks + w in 8 chunks (4 groups each) on the Sync HWDGE
    queue; y^T written per 4 groups on the Scalar HWDGE queue so output
    never queues behind input loads.  All lines >= 2 KB.
  - Host: pre-transposes x into partition-major per-core layout, packs
    transposed weight blocks into a lane-major array, assembles y.
"""

import numpy as np
import ml_dtypes

BF16 = ml_dtypes.bfloat16

# toggles used by test.py only; harness uses defaults
_RUN = {"trace": False, "trace_cores": [0], "last": None}

B, K, OUT, BLK, NNZ = 2048, 4096, 4096, 32, 2048
NCORES = 8
BC = B // NCORES          # 256 batch rows per core
NT = K // 128             # 32 x^T partition-tiles
NRB = OUT // BLK          # 128 row blocks
NG = NRB // 4             # 32 groups of 4 row blocks
GC = 4                    # groups per w-DMA chunk / y-DMA batch
NWC = NG // GC            # 8 w chunks
XCH = 4                   # x chunks


def _build_schedule(w, rows, cols):
    """Group assignment + per-(group, lane) slot schedule + packed weights."""
    cnt = np.bincount(rows, minlength=NRB)
    order = np.argsort(-cnt, kind="stable")
    rmap = np.empty((NG, 4), dtype=np.int64)
    for rank, r in enumerate(order):
        rnd, pos = rank // NG, rank % NG
        g = pos if rnd % 2 == 0 else NG - 1 - pos
        rmap[g, rnd] = r

    gb_of_row = {}
    for g in range(NG):
        for b in range(4):
            gb_of_row[int(rmap[g, b])] = (g, b)

    cells = [[[[] for _ in range(4)] for _ in range(4)] for _ in range(NG)]
    for n in range(NNZ):
        g, b = gb_of_row[int(rows[n])]
        cells[g][int(cols[n]) % 4][b].append(n)

    # prog[g][a] = list of slots (t, b, start, stop, wT[32,32]), sorted by
    # x-tile index t so matmuls become eligible as x chunks stream in.
    prog = []
    for g in range(NG):
        lanes = []
        for a in range(4):
            raw = []
            for b in range(4):
                cl = cells[g][a][b]
                if not cl:
                    raw.append((0, b, np.zeros((BLK, BLK), np.float32)))
                for n in cl:
                    raw.append((int(cols[n]) // 4, b,
                                np.ascontiguousarray(w[n].T)))
            raw.sort(key=lambda s: s[0])
            # interleave b's: consecutive same-(a,b) slots hit the same PE
            # subarray and serialize; pick the next slot with a different b
            # from a small lookahead window (keeps t within one x-chunk)
            reordered = []
            pend = list(raw)
            prevb = None
            while pend:
                pick = 0
                for j in range(min(6, len(pend))):
                    if pend[j][1] != prevb and pend[j][0] <= pend[0][0] + 2:
                        pick = j
                        break
                slot = pend.pop(pick)
                reordered.append(slot)
                prevb = slot[1]
            raw = reordered
            first = {}
            last = {}
            for i, (_, b, _) in enumerate(raw):
                first.setdefault(b, i)
                last[b] = i
            slots = [(t, b, i == first[b], i == last[b], wt)
                     for i, (t, b, wt) in enumerate(raw)]
            lanes.append(slots)
        prog.append(lanes)

    offs, tot = [], 0
    for g in range(NG):
        offs.append(tot)
        tot += max(len(prog[g][a]) for a in range(4))
    wpk = np.zeros((128, tot * BLK), dtype=np.float32)
    for g in range(NG):
        for a in range(4):
            for idx, (_, _, _, _, wt) in enumerate(prog[g][a]):
                col = (offs[g] + idx) * BLK
                wpk[32 * a:32 * a + 32, col:col + BLK] = wt
    return prog, offs, tot, wpk, rmap


def kernel(x, w, rows, cols, out_blocks=None):
    import concourse.bass as bass
    import concourse.bacc as bacc
    import concourse.tile as tile
    import concourse.mybir as mybir
    from concourse.bass_utils import run_bass_kernel_spmd
    from contextlib import ExitStack

    x = np.asarray(x, dtype=np.float32)
    w = np.asarray(w, dtype=np.float32)
    rows = np.asarray(rows).astype(np.int64)
    cols = np.asarray(cols).astype(np.int64)

    prog, offs, tot, wpk, rmap = _build_schedule(w, rows, cols)
    offs4 = [offs[k * GC] for k in range(NWC)] + [tot]
    wpk16 = wpk.astype(BF16)

    # x^T, per-core partition-major: xarr[core, p, t*BC + j] = x[BC*core + j, 128*t + p]
    xarr = np.ascontiguousarray(
        x.reshape(NCORES, BC, NT, 128).transpose(0, 3, 2, 1)
    ).reshape(NCORES, 128, NT * BC).astype(BF16)

    f32 = mybir.dt.float32
    bf16 = mybir.dt.bfloat16
    nc = bacc.Bacc()
    xt_d = nc.declare_dram_parameter("xt", [128, NT * BC], bf16, isOutput=False)
    wp_d = nc.declare_dram_parameter("wpk", [128, tot * BLK], bf16, isOutput=False)
    yt_d = nc.declare_dram_parameter("yt", [128, NG * BC], bf16, isOutput=True)

    with tile.TileContext(nc) as tc, ExitStack() as ctx:
        xp = ctx.enter_context(tc.tile_pool(name="x", bufs=1))
        wpool = ctx.enter_context(tc.tile_pool(name="w", bufs=3))
        pp = ctx.enter_context(tc.tile_pool(name="ps", bufs=8, space="PSUM"))
        tp = ctx.enter_context(tc.tile_pool(name="tmp", bufs=2))
        yp = ctx.enter_context(tc.tile_pool(name="y", bufs=2))

        wtiles = {}

        def load_w(k):
            ncols = (offs4[k + 1] - offs4[k]) * BLK
            wsb = wpool.tile([128, ncols], bf16, tag="w", name=f"w{k}")
            nc.sync.dma_start(
                wsb[:], wp_d[:, offs4[k] * BLK:offs4[k + 1] * BLK])
            wtiles[k] = wsb

        XC = NT // XCH
        xts = []

        def load_x(ci):
            xc = xp.tile([128, XC * BC], bf16, tag=f"xc{ci}", name=f"xc{ci}")
            nc.sync.dma_start(
                xc[:], xt_d[:, ci * XC * BC:(ci + 1) * XC * BC])
            xts.append(xc)

        # DMA ring is FIFO: first w chunk and first x chunk lead the queue.
        load_w(0)
        load_x(0)
        load_w(1)
        for ci in range(1, XCH):
            load_x(ci)

        def rhs_of(t):
            return xts[t // XC][:, (t % XC) * BC:(t % XC + 1) * BC]

        y4 = None
        for g in range(NG):
            k = g // GC
            if g % GC == 0 and k + 2 < NWC:
                load_w(k + 2)
            wsb = wtiles[k]
            wbase = offs[g] - offs4[k]
            n_g = max(len(prog[g][a]) for a in range(4))
            ps = [pp.tile([128, BC], f32, tag="ps", name=f"ps{a}")
                  for a in range(4)]
            for idx in range(n_g):
                for a in range(4):
                    if idx < len(prog[g][a]):
                        t, b, st, sp, _ = prog[g][a][idx]
                        wcol = (wbase + idx) * BLK
                        nc.tensor.matmul(
                            ps[a][32 * b:32 * b + 32, :],
                            lhsT=wsb[32 * a:32 * a + 32, wcol:wcol + BLK],
                            rhs=rhs_of(t)[32 * a:32 * a + 32, :],
                            start=st, stop=sp,
                            tile_position=(32 * a, 32 * b),
                        )
            # PSUM read ports: ACT evacuates two banks, DVE folds two more
            # (one PSUM operand per DVE op), Pool does the SBUF-only final
            # add with the bf16 downcast.
            s0 = tp.tile([128, BC], f32, tag="t0")
            nc.scalar.copy(s0[:], ps[0][:])
            s2 = tp.tile([128, BC], f32, tag="t1")
            nc.scalar.copy(s2[:], ps[2][:])
            a01 = tp.tile([128, BC], f32, tag="t2")
            nc.vector.tensor_add(a01[:], s0[:], ps[1][:])
            a23 = tp.tile([128, BC], f32, tag="t3")
            nc.vector.tensor_add(a23[:], s2[:], ps[3][:])
            if g % GC == 0:
                y4 = yp.tile([128, GC * BC], bf16, tag="y")
            nc.gpsimd.tensor_add(
                y4[:, (g % GC) * BC:(g % GC + 1) * BC], a01[:], a23[:])
            if g % GC == GC - 1:
                nc.scalar.dma_start(
                    yt_d[:, (g - GC + 1) * BC:(g + 1) * BC], y4[:])

    nc.compile()

    in_maps = [{"xt": xarr[i], "wpk": wpk16} for i in range(NCORES)]
    res = run_bass_kernel_spmd(
        nc, in_maps, list(range(NCORES)),
        trace=_RUN["trace"], trace_cores=_RUN["trace_cores"],
    )
    _RUN["last"] = res

    feat = np.empty(OUT, dtype=np.int64)
    for g in range(NG):
        for b in range(4):
            feat[128 * g + 32 * b:128 * g + 32 * b + 32] = \
                32 * rmap[g, b] + np.arange(32)

    y = np.empty((B, OUT), dtype=np.float32)
    for i in range(NCORES):
        ytp = np.asarray(res.results[i]["yt"]).astype(np.float32)
        ytp = ytp.reshape(128, NG, BC).transpose(1, 0, 2).reshape(OUT, BC)
        yT = np.empty((OUT, BC), dtype=np.float32)
        yT[feat] = ytp
        y[BC * i:BC * (i + 1), :] = yT.T
    return y
